# revision 5
# baseline (speedup 1.0000x reference)
# MoE layer (8 experts, top-2) on 8 TRN2 NeuronCores.
#
# Strategy: expert parallelism (core e owns expert e), per the sharding hint.
#   * Host (control plane): computes gate routing decisions, dispatches
#     ("all-to-all") each token's row to the core(s) owning its top-2 experts,
#     and combines the per-expert partial outputs back into the full output.
#   * Device (data plane): for each core e, computes
#         yT = sigmoid(dlg) * ( W2[e].T @ gelu( W1[e].T @ xT ) )
#     where xT is the (C x CAP) gathered token block for expert e (transposed
#     so the contraction dim lives on SBUF partitions), and sigmoid(dlg) is
#     exactly the top-2 softmax weight for the owning expert
#     (softmax([a,b])[0] == sigmoid(a-b)).
#
# Precision/throughput: both matmuls run on the PE array in fp8-e4m3 with
# MatmulPerfMode.DoubleRow (2 stationary rows per PE cell, 0.5 cycles per
# output row => 4x bf16 MAC throughput). Plain fp8 rounding is too coarse
# for the 2e-2 gate, so every operand is carried as a hi + lo pair
# (lo = fp8 of the quantization residual) and each product is computed as
#     hi.hi  +  lo.hi  +  hi.lo        (lo.lo dropped)
# All three terms share one power-of-2 scale, so they accumulate into a
# single fp32 PSUM group; measured end-to-end rel-err 1.8e-3 (vs bf16's
# 3.4e-3) at 0.75x the bf16 PE cost.
#
# Layout (contraction dim on partitions, tokens on the moving free dim):
#   x  dram [P, CO, 2, cap]   fp8, dim2 = (hi, lo), values 8*x
#   w1 dram [P, CO, 2, F]     fp8, dim2 = (lo, hi), values 64*W1
#   w2 dram [P, CO, FO, 2, P] fp8, dim2 = (lo, hi), values 256*W2
#   phase 1:  ps1 = 512*(W1.T @ x)  -> h = gelu(ps1/512), stored hi/lo fp8
#   phase 2:  ps2 = 256*(W2.T @ h)  -> y = ps2 * (sigmoid(dlg)/256)
# Main terms pair ADJACENT contraction chunks in one DoubleRow instruction;
# cross terms pair (W_lo, x_hi) + (W_hi, x_lo) of the SAME chunk in one.
# W1 stays resident in SBUF; W2 streams per token block; tokens stream in
# blocks of 512 (the PSUM-bank moving-dim limit for fp32 accumulation).

import math

import numpy as np
import ml_dtypes

import concourse.bass as bass
import concourse.mybir as mybir
import concourse.tile as tile
from concourse import bacc
from concourse.bass_utils import run_bass_kernel_spmd

C = 1024          # d_model
F = 4096          # d_ff
E = 8             # experts == cores
P = 128           # SBUF partitions
NTOK = 512        # moving-dim token block (one PSUM bank of fp32)
CO = C // P       # 8 contraction chunks, phase 1
FO = F // P       # 32 contraction chunks, phase 2
SX = 8.0          # fp8 scale on x
SW1 = 64.0        # fp8 scale on W1   (psum1 = SX*SW1 * z = 512 z)
SW2 = 256.0       # fp8 scale on W2   (h at scale 1; psum2 = 256 * (h@W2))
FP8 = mybir.dt.float8e4
F32 = mybir.dt.float32
DR = mybir.MatmulPerfMode.DoubleRow
E4M3 = ml_dtypes.float8_e4m3fn

# Filled by kernel() on each call, for the test harness to inspect.
last_run_info: dict = {}

# NEFF-module memo: cap -> compiled Bass module (routing is deterministic in
# the inputs, so repeat calls reuse the same module and its cached NEFF).
_nc_cache: dict = {}


def _build_ffn(cap: int, ntok: int = NTOK) -> bass.Bass:
    """Per-core expert-FFN kernel, fp8 DoubleRow with hi/lo error correction."""
    act_fn = mybir.ActivationFunctionType.Gelu
    nc = bacc.Bacc()

    xt = nc.dram_tensor("xt", [P, CO, 2, cap], FP8, kind="ExternalInput")
    w1 = nc.dram_tensor("w1", [P, CO, 2, F], FP8, kind="ExternalInput")
    w2 = nc.dram_tensor("w2", [P, CO, FO, 2, P], FP8, kind="ExternalInput")
    dlg = nc.dram_tensor("dlg", [P, cap], F32, kind="ExternalInput")
    yt = nc.dram_tensor("yt", [C, cap], F32, kind="ExternalOutput")

    yt_r = yt.rearrange("(co ci) t -> ci co t", ci=P)

    with tile.TileContext(nc) as tc:
        with (
            tc.tile_pool(name="wts", bufs=1) as wpool,
            tc.tile_pool(name="w2s", bufs=3) as w2pool,
            tc.tile_pool(name="xts", bufs=2) as xpool,
            tc.tile_pool(name="hts", bufs=1) as hpool,
            tc.tile_pool(name="g32s", bufs=2) as gpool,
            tc.tile_pool(name="ces", bufs=2) as cepool,
            tc.tile_pool(name="yts", bufs=3) as ypool,
            tc.tile_pool(name="ps", bufs=4, space="PSUM") as pspool,
        ):
            # Block 0's token DMAs are issued BEFORE the w1 load: the DMA
            # queue is FIFO, and the first matmul needs xt.
            xt0 = xpool.tile([P, CO, 2, ntok], FP8, tag="xt")
            t0n = min(ntok, cap)
            nc.sync.dma_start(xt0[:, :, :, :t0n], xt[:, :, :, :t0n])

            # Resident w1 (hi+lo fp8, 64 KiB/partition), loaded in f-major
            # chunks so phase 1's fo-th psum group only waits for the chunk
            # covering it (finer chunks up front so the first psum group
            # starts ~4 us after launch). w2 streams per token block.
            w1_sb = wpool.tile([P, CO, 2, F], FP8, tag="w1")
            f0 = 0
            for fch in (256, 256, 512, 1024, 1024, 1024):
                for co in range(CO):
                    nc.sync.dma_start(
                        w1_sb[:, co, :, f0 : f0 + fch], w1[:, co, :, f0 : f0 + fch]
                    )
                f0 += fch
            assert f0 == F

            nblk = (cap + ntok - 1) // ntok
            for b in range(nblk):
                t0 = b * ntok
                tn = min(ntok, cap - t0)

                if b == 0:
                    xt_t = xt0
                else:
                    xt_t = xpool.tile([P, CO, 2, ntok], FP8, tag="xt")
                    nc.sync.dma_start(
                        xt_t[:, :, :, :tn], xt[:, :, :, t0 : t0 + tn]
                    )
                # Combine weight ce = sigmoid(dlg)/SW2, via
                # sigmoid(z) = 0.5*tanh(z/2) + 0.5 (tanh shares an ACT table
                # with gelu; sigmoid does not).
                dlg_t = cepool.tile([P, ntok], F32, tag="dlg")
                nc.sync.dma_start(dlg_t[:, :tn], dlg[:, t0 : t0 + tn])
                ce_t = cepool.tile([P, ntok], F32, tag="ce")
                nc.scalar.activation(
                    ce_t[:, :tn], dlg_t[:, :tn],
                    mybir.ActivationFunctionType.Tanh, scale=0.5,
                )
                nc.vector.tensor_scalar(
                    ce_t[:, :tn], ce_t[:, :tn], 0.5 / SW2, 0.5 / SW2,
                    mybir.AluOpType.mult, mybir.AluOpType.add,
                )

                # Phase 1: ps1 = 512*(W1.T @ x) for this token block, then
                # h = gelu(ps1/512) split into hi (fp8) + lo (fp8 residual).
                # ht is 4 sub-tiles of 8 fo-chunks each so phase 2's psum
                # groups only wait on the quarter of phase 1 they read, not
                # the full ACT/DVE tail.
                FQ = FO // 4
                ht_q = [
                    hpool.tile([P, FQ, 2, ntok], FP8, tag=f"ht{q}", name=f"ht{q}")
                    for q in range(4)
                ]
                for fo in range(FO):
                    col = slice(fo * P, (fo + 1) * P)
                    hq, hf = ht_q[fo // FQ], fo % FQ
                    ps = pspool.tile([P, ntok], F32, tag="ps")
                    for j in range(CO // 2):
                        nc.tensor.matmul(
                            ps[:, :tn],
                            w1_sb[:, 2 * j : 2 * j + 2, 1, col],
                            xt_t[:, 2 * j : 2 * j + 2, 0, :tn],
                            start=(j == 0), stop=False, perf_mode=DR,
                        )
                    for co in range(CO):
                        nc.tensor.matmul(
                            ps[:, :tn],
                            w1_sb[:, co, :, col],
                            xt_t[:, co, :, :tn],
                            start=False, stop=(co == CO - 1), perf_mode=DR,
                        )
                    nc.scalar.activation(
                        hq[:, hf, 0, :tn], ps[:, :tn], act_fn, scale=1.0 / 512
                    )
                    g32 = gpool.tile([P, ntok], F32, tag="g32")
                    nc.scalar.activation(
                        g32[:, :tn], ps[:, :tn], act_fn, scale=1.0 / 512
                    )
                    nc.vector.tensor_tensor(
                        hq[:, hf, 1, :tn], g32[:, :tn], hq[:, hf, 0, :tn],
                        mybir.AluOpType.subtract,
                    )

                # Phase 2: ps2 = 256*(W2.T @ h); y = ps2 * ce.
                for co in range(CO):
                    w2_t = w2pool.tile([P, FO, 2, P], FP8, tag="w2s")
                    nc.sync.dma_start(w2_t[:], w2[:, co])
                    ps2 = pspool.tile([P, ntok], F32, tag="ps")
                    for j in range(FO // 2):
                        fo = 2 * j
                        nc.tensor.matmul(
                            ps2[:, :tn],
                            w2_t[:, fo : fo + 2, 1, :],
                            ht_q[fo // FQ][:, fo % FQ : fo % FQ + 2, 0, :tn],
                            start=(j == 0), stop=False, perf_mode=DR,
                        )
                    for fo in range(FO):
                        nc.tensor.matmul(
                            ps2[:, :tn],
                            w2_t[:, fo, :, :],
                            ht_q[fo // FQ][:, fo % FQ, :, :tn],
                            start=False, stop=(fo == FO - 1), perf_mode=DR,
                        )
                    y_t = ypool.tile([P, ntok], F32, tag="y")
                    nc.vector.tensor_tensor(
                        y_t[:, :tn], ps2[:, :tn], ce_t[:, :tn],
                        mybir.AluOpType.mult,
                    )
                    nc.sync.dma_start(yt_r[:, co, t0 : t0 + tn], y_t[:, :tn])

    # bacc passes: register allocation, and crucially generate_event_semaphores,
    # which splits multi-wait sync conditions (HW allows 1 wait per instruction).
    nc.compile()

    # Guard: the Tile allocator believes SBUF is 224 KiB/partition (the ISA
    # constant), but exceeding ~192 KiB crashes the TRN2 exec unit. Keep a
    # hard ceiling so overflows fail at build time, not on silicon.
    hw = 0
    for alloc in nc.to_json()["functions"][0]["allocations"]:
        for ml in alloc.get("memorylocations") or []:
            if ml.get("type") == "SB":
                hw = max(hw, ml["addr"] + ml["dims"][1])
    assert hw <= 184 * 1024, f"SBUF high-water {hw / 1024:.1f} KiB exceeds 184 KiB"
    return nc


def _gate_jax_cpu(xf: np.ndarray, Wg: np.ndarray):
    """Reproduce the reference's gate bit-exactly: fp32 matmul + lax.top_k
    on the jax CPU backend (including its tie-breaking). Falls back to a
    numpy gate (correct except possibly on exact fp32 knife-edge ties) if
    jax is unavailable."""
    try:
        import jax

        cpu = jax.devices("cpu")[0]
        with jax.default_device(cpu):
            logits = jax.device_put(xf, cpu) @ jax.device_put(Wg, cpu)
            tv, ti = jax.lax.top_k(logits, 2)
            return np.asarray(ti), np.asarray(tv)
    except Exception:
        logits = xf @ Wg
        part = np.argpartition(-logits, 1, axis=1)[:, :2]
        pv = np.take_along_axis(logits, part, axis=1)
        order = np.argsort(-pv, axis=1, kind="stable")
        ti = np.take_along_axis(part, order, axis=1)
        tv = np.take_along_axis(logits, ti, axis=1)
        return ti, tv


def _split8(v: np.ndarray):
    """fp8-e4m3 hi/lo decomposition: hi = q(v), lo = q(v - hi)."""
    hi = v.astype(E4M3)
    lo = (v - hi.astype(np.float32)).astype(E4M3)
    return hi, lo


def kernel(x, Wg, W1, W2):
    x = np.asarray(x, dtype=np.float32)
    Wg = np.asarray(Wg, dtype=np.float32)
    W1 = np.asarray(W1, dtype=np.float32)
    W2 = np.asarray(W2, dtype=np.float32)

    B, T, _ = x.shape
    N = B * T
    xf = x.reshape(N, C)

    # ---- Gate + routing (control plane) ----
    # Routing decisions are knife-edge sensitive: for this problem one token
    # has a 2.7e-7 gap between its 2nd and 3rd expert logits, smaller than
    # fp32 GEMM rounding differences between BLAS implementations. Compute
    # the gate with the same jax-on-CPU ops the reference uses so the top-2
    # selection matches it bit-for-bit.
    top2, tv = _gate_jax_cpu(xf, Wg)                        # (N, 2) ids / logits

    sels = []
    counts = []
    for e in range(E):
        sel = np.nonzero((top2 == e).any(axis=1))[0]
        sels.append(sel)
        counts.append(len(sel))
    # cap needs no partition alignment — tokens are the free dim everywhere.
    # Round to mult of 4 so fp8 rows stay 4-byte aligned.
    cap = max(NTOK, math.ceil(max(counts) / 4) * 4)

    # ---- Token dispatch (all-to-all equivalent) ----
    in_maps = []
    for e in range(E):
        sel = sels[e]
        cnt = len(sel)
        row = top2[sel]
        tvr = tv[sel]
        own = np.where(row[:, 0] == e, tvr[:, 0], tvr[:, 1])
        other = np.where(row[:, 0] == e, tvr[:, 1], tvr[:, 0])

        xg = (SX * xf[sel].T).reshape(CO, P, cnt).transpose(1, 0, 2)
        xh, xl = _split8(xg)
        xt = np.zeros((P, CO, 2, cap), dtype=E4M3)
        xt[:, :, 0, :cnt] = xh
        xt[:, :, 1, :cnt] = xl

        dlg = np.full((cap,), -60.0, dtype=np.float32)
        dlg[:cnt] = own - other
        dlg_b = np.ascontiguousarray(
            np.broadcast_to(dlg[None, :], (P, cap)), dtype=np.float32
        )

        w1h, w1l = _split8((SW1 * W1[e]).reshape(CO, P, F).transpose(1, 0, 2))
        w1s = np.ascontiguousarray(np.stack([w1l, w1h], axis=2))  # [P,CO,2,F]

        v2 = (SW2 * W2[e]).reshape(FO, P, CO, P)
        w2h, w2l = _split8(v2)
        # [fo, fi, co, cc] x2 -> [fi, co, fo, 2, cc]
        w2s = np.ascontiguousarray(
            np.stack([w2l, w2h], axis=0).transpose(2, 3, 1, 0, 4)
        )
        in_maps.append({"xt": xt, "w1": w1s, "w2": w2s, "dlg": dlg_b})

    # ---- Expert FFN on the 8 NeuronCores ----
    nc = _nc_cache.get(cap)
    if nc is None:
        nc = _nc_cache[cap] = _build_ffn(cap)
    res = run_bass_kernel_spmd(nc, in_maps, core_ids=list(range(E)))

    global last_run_info
    last_run_info = {
        "cap": cap,
        "counts": counts,
        "exec_time_ns": res.exec_time_ns,
        "mean_exec_time_ns": res.mean_exec_time_ns,
        "instructions_and_trace": res.instructions_and_trace,
        "profile_json": res.profile_json,
    }

    # ---- Combine (weighted scatter-add) ----
    out = np.zeros((N, C), dtype=np.float32)
    for e in range(E):
        sel = sels[e]
        out[sel] += res.results[e]["yt"][:, : len(sel)].T
    return out.reshape(B, T, C)
